# revision 29
# baseline (speedup 1.0000x reference)
"""AttentivePool Trainium2 kernel (v7 - asymmetric q, PSUM-conflict-free pipeline).

Reference computation per sample (x [C,T], prefix mask m [T]):
  stats1: mu/sd of x over valid frames
  h = w1a@x + (w1b@mu + w1c@sd + b1)  -> LayerNorm over 128 ch -> relu -> tanh
  a = w2@th (+b2), softmax over valid t, weighted mu2/sd2 of x -> [2C]

Sharding: pure data parallel, 2 samples per core across 8 cores.

Strategy:
  - x loaded twice from HBM: natural [C,T] chunks (mm1 rhs only) and
    host-prepped xm = (x.T * m/L) blocks [128t, C] bf16 (everything else).
    exp(a - 30*(1-m)) ~ 0 exactly where xm = 0, so the nm-weighting of xm
    is harmless for the softmax moments and folds out as L / L^2 scalings
    in the tiny phase-5 ops.
  - all T-direction reductions are PE matvecs vs a ones column (out free
    size 1 ~ free in the cost model): mu, q, den, sex', sx2'
  - q via x2m = xm*xm on DVE (hidden under the DMA ramp) + PE matvecs
  - mask enters phase 4 as a -30 per-partition bias column on the Exp
  - LayerNorm via bf16 PE transposes, pipelined per T-half: tanh fused
    with the normalize (Tanh(rstd*h - mean*rstd) on ACT); the PSUM->SBUF
    copy of the back-transpose doubles as the relu (max(tanh,0))
  - phase 4 per T-block: aT = th_blk^T @ w2T on PE (3 bank matmuls),
    whole-block Exp on ACT, p1 = e*xm / p2 = p1*xm on DVE (bf16 2x, some
    p2 on GPSIMD), 36 matvec reductions on PE lagged 2 blocks
  - engines are in-order with shallow wait queues, so sample 1's front
    (mm1, x2m, mu/q matvecs, LN) is emitted INSIDE sample 0's phase-4
    loop at slots matched to its DMA arrivals
  - PSUM 8 banks: AU tag 3 banks x2 bufs (LN transposes then aT blocks),
    mm1 quarter tag 1 bank, acc 1 bank (per-sample column ranges)
"""

import numpy as np

B, C, T = 16, 1536, 2000
ATTN = 128
NCORES = 8
BPC = B // NCORES          # samples per core
NCH = C // 128             # 12 channel chunks
TPAD = 2048                # padded T for 16x128 transpose blocks
NTT = TPAD // 128          # 16 T blocks
MASK_NEG = -30.0

POOL_COPY = False           # one of two th copies per sample on GPSIMD
POOL_P2_EVERY = 0          # every k-th block's p2 on GPSIMD (0 = off)

_CACHE = {}


def _split_waits(nc, max_waits=1):
    """walrus in this toolchain rejects >1 sync-wait per instruction; hoist
    excess waits onto injected same-engine NOPs that run just before."""
    from concourse import mybir
    ctr = 0
    for fn in nc.m.functions:
        for blk in fn.blocks:
            out = []
            changed = False
            for ins in blk.instructions:
                si = ins.sync_info
                ow = list(si.on_wait) if si and si.on_wait else []
                if len(ow) > max_waits:
                    changed = True
                    excess = ow[:-max_waits]
                    for i in range(0, len(excess), max_waits):
                        ctr += 1
                        out.append(mybir.InstNoOp(
                            name=f"wsplit_{ctr}", engine=ins.engine,
                            ins=[], outs=[],
                            sync_info=mybir.SyncInfo(
                                on_wait=excess[i:i + max_waits],
                                on_update=[])))
                    si.on_wait = ow[-max_waits:]
                    ins.sync_info = si
                out.append(ins)
            if changed:
                blk.instructions = out


def _build_nc(trivial_ln=True, trivial_b2=True):
    import concourse.bass as bass
    import concourse.tile as tile
    from concourse import mybir

    fp32 = mybir.dt.float32
    bf16 = mybir.dt.bfloat16
    AF = mybir.ActivationFunctionType
    OP = mybir.AluOpType

    nc = bass.Bass("TRN2", target_bir_lowering=False)

    x_d = nc.dram_tensor("x", [BPC, C, T], mybir.dt.float8e4,
                         kind="ExternalInput")
    xm_d = nc.dram_tensor("xm", [BPC, NTT, 128, C], bf16, kind="ExternalInput")
    mbT_d = nc.dram_tensor("mbT", [BPC, 128, NTT], fp32, kind="ExternalInput")
    scal_d = nc.dram_tensor("scal", [BPC, 128, 2], fp32, kind="ExternalInput")
    w1aT_d = nc.dram_tensor("w1aT", [128, NCH, 128],
                            mybir.dt.float8e4, kind="ExternalInput")
    w1bT_d = nc.dram_tensor("w1bT", [128, NCH, 128], bf16, kind="ExternalInput")
    w1cT_d = nc.dram_tensor("w1cT", [128, NCH, 128], bf16, kind="ExternalInput")
    w2T_d = nc.dram_tensor("w2T", [128, C], bf16, kind="ExternalInput")
    ones_d = nc.dram_tensor("ones_col", [128, 1], bf16, kind="ExternalInput")
    id_d = nc.dram_tensor("identb", [128, 128], bf16, kind="ExternalInput")
    cst_d = nc.dram_tensor("cst", [128, 16], fp32, kind="ExternalInput")
    gb_d = nc.dram_tensor("gb", [128, 128], fp32, kind="ExternalInput")
    bb_d = nc.dram_tensor("bb", [128, 128], fp32, kind="ExternalInput")
    ones1_d = nc.dram_tensor("ones1", [1, 128], bf16, kind="ExternalInput")
    b2r_d = nc.dram_tensor("b2r", [1, C], bf16, kind="ExternalInput")
    out_d = nc.dram_tensor("out", [BPC, 2 * C], fp32, kind="ExternalOutput")

    with tile.TileContext(nc) as tc:
        import contextlib
        with contextlib.ExitStack() as ctx:
            consts = ctx.enter_context(tc.tile_pool(name="consts", bufs=1))
            xmp = ctx.enter_context(tc.tile_pool(name="xmp", bufs=2))
            xpool = ctx.enter_context(tc.tile_pool(name="xres", bufs=1))
            hp = ctx.enter_context(tc.tile_pool(name="hbuf", bufs=1))
            thp = ctx.enter_context(tc.tile_pool(name="thp", bufs=1))
            scr = ctx.enter_context(tc.tile_pool(name="scr", bufs=3))
            stp = ctx.enter_context(tc.tile_pool(name="stats", bufs=2))
            pAU = ctx.enter_context(tc.tile_pool(name="pAU", bufs=2,
                                                 space="PSUM"))
            pQ = ctx.enter_context(tc.tile_pool(name="pQ", bufs=1,
                                                space="PSUM"))
            pacc = ctx.enter_context(tc.tile_pool(name="pacc", bufs=1,
                                                  space="PSUM"))

            # --- constants (loaded once) ---
            fp8 = mybir.dt.float8e4
            w1aT = consts.tile([128, NCH, 128], fp8)
            w1bT = consts.tile([128, NCH, 128], bf16)
            w1cT = consts.tile([128, NCH, 128], bf16)
            w2T = consts.tile([128, C], bf16)
            ones_col = consts.tile([128, 1], bf16)
            identb = consts.tile([128, 128], bf16)
            cst = consts.tile([128, 16], fp32)
            const_loads = [(w1aT, w1aT_d), (w1bT, w1bT_d), (w1cT, w1cT_d),
                           (w2T, w2T_d), (ones_col, ones_d), (identb, id_d),
                           (cst, cst_d)]
            gb = bb = ones1 = b2r = None
            if not trivial_ln:
                gb = consts.tile([128, 128], fp32)
                bb = consts.tile([128, 128], fp32)
                const_loads += [(gb, gb_d), (bb, bb_d)]
            if not trivial_b2:
                ones1 = consts.tile([1, 128], bf16)
                b2r = consts.tile([1, C], bf16)
                const_loads += [(ones1, ones1_d), (b2r, b2r_d)]
            critical = {"w1aT", "ones_col", "identb", "cst"}
            for t_, d_ in const_loads:
                if d_.name in critical:
                    nc.sync.dma_start(out=t_, in_=d_[:])
            late_consts = [(t_, d_) for t_, d_ in const_loads
                           if d_.name not in critical]
            b1c = cst[:, 0:1]
            eps_c = cst[:, 13:14]

            # acc bank: per-sample column ranges (96*s offset):
            #   mu 0:12 | q 16:28 | den 32:44 | sex 44:56 | sx2 56:68 | cb 88
            acc = pacc.tile([128, 192], fp32, tag="acc")
            nc.vector.memset(acc, 0.0)
            ACC_MM = dict(start=False, stop=False, skip_group_check=True)
            S = {}

            def emit_dmas(s):
                st = S[s] = {}
                st["mb"] = stp.tile([128, NTT], fp32, tag="mb", name="mb")
                st["sc"] = stp.tile([128, 2], fp32, tag="sc", name="sc")
                nc.sync.dma_start(out=st["mb"], in_=mbT_d[s])
                nc.sync.dma_start(out=st["sc"], in_=scal_d[s])
                def load_x():
                    st["xc"] = []
                    for i in range(NCH):
                        xi = xpool.tile([128, T], fp8, tag=f"x{i}",
                                        name="xi")
                        st["xc"].append(xi)
                        nc.sync.dma_start(
                            out=xi, in_=x_d[s, i * 128:(i + 1) * 128, :])

                def load_xm():
                    st["xm"] = []
                    for b in range(NTT):
                        xb = xmp.tile([128, C], bf16, tag=f"xm{b}",
                                      name="xmb", bufs=2)
                        st["xm"].append(xb)
                        nc.sync.dma_start(out=xb, in_=xm_d[s, b])

                if s == 0:
                    load_x()
                    for t_, d_ in late_consts:
                        nc.sync.dma_start(out=t_, in_=d_[:])
                    load_xm()
                else:
                    load_xm()
                    load_x()

            def emit_fA(s):
                """mm1 in four T-quarters on the 1-bank PSUM tag; raw
                (bias-free) PSUM->SBUF quarter copies so the rotation drains
                at mm1 speed."""
                st = S[s]
                hsb = hp.tile([128, TPAD], bf16, tag="hsb", name="hsb")
                st["hsb"] = hsb
                for qt in range(4):
                    qo = qt * 512
                    n = min(512, T - qo)
                    hps = pQ.tile([128, 512], fp32, tag="Q", name="hps")
                    for i in range(NCH):
                        nc.tensor.matmul(
                            hps[:, 0:n], lhsT=w1aT[:, i, :],
                            rhs=st["xc"][i][:, qo:qo + n],
                            start=(i == 0), stop=(i == NCH - 1))
                    if qt % 2 == 0:
                        nc.scalar.activation(out=hsb[:, qo:qo + n],
                                             in_=hps[:, 0:n], func=AF.Copy)
                    else:
                        nc.vector.tensor_scalar(out=hsb[:, qo:qo + n],
                                                in0=hps[:, 0:n], scalar1=1.0,
                                                scalar2=None, op0=OP.mult)

            def emit_fB(s, b):
                """x2m on DVE + 24 mu/q matvecs on PE for block b."""
                st = S[s]
                ao = 96 * s
                x2m = scr.tile([128, C], bf16, tag="x2m", name="x2m",
                               bufs=2)
                nc.vector.tensor_mul(out=x2m, in0=st["xm"][b],
                                     in1=st["xm"][b])
                for i in range(NCH):
                    cs = slice(i * 128, (i + 1) * 128)
                    nc.tensor.matmul(acc[:, ao + i:ao + i + 1],
                                     lhsT=st["xm"][b][:, cs], rhs=ones_col,
                                     **ACC_MM)
                    nc.tensor.matmul(acc[:, ao + 16 + i:ao + 17 + i],
                                     lhsT=x2m[:, cs], rhs=ones_col, **ACC_MM)

            def emit_fC(s):
                """mu/sd/cb, h bias, LN -> tanh -> relu-copy -> th."""
                st = S[s]
                ao = 96 * s
                Lc = st["sc"][:, 0:1]
                hsb = st["hsb"]

                sv = stp.tile([128, 272], fp32, tag="sv", name="sv")
                st["sv"] = sv
                var_a = sv[:, 24:36]
                t0_a = sv[:, 96:108]
                st6 = sv[:, 112:208].rearrange("p (j k) -> p j k", k=6)
                mv = sv[:, 208:240].rearrange("p (j two) -> p j two", two=2)
                rstd = sv[:, 240:256]
                nmr = sv[:, 256:272]
                svb = stp.tile([128, 32], bf16, tag="svb", name="svb")
                mu_b = svb[:, 0:12]
                sd_b = svb[:, 12:24]

                nc.vector.tensor_copy(out=mu_b, in_=acc[:, ao:ao + 12])
                nc.vector.tensor_mul(out=t0_a, in0=mu_b, in1=mu_b)
                nc.vector.scalar_tensor_tensor(
                    out=var_a, in0=acc[:, ao + 16:ao + 28], scalar=Lc,
                    in1=t0_a, op0=OP.mult, op1=OP.subtract)
                nc.vector.tensor_scalar(out=var_a, in0=var_a, scalar1=1e-9,
                                        scalar2=None, op0=OP.max)
                nc.scalar.activation(out=sd_b, in_=var_a, func=AF.Sqrt)
                for i in range(NCH):
                    nc.tensor.matmul(acc[:, ao + 88:ao + 89],
                                     lhsT=w1bT[:, i, :],
                                     rhs=mu_b[:, i:i + 1], **ACC_MM)
                for i in range(NCH):
                    nc.tensor.matmul(acc[:, ao + 88:ao + 89],
                                     lhsT=w1cT[:, i, :],
                                     rhs=sd_b[:, i:i + 1], **ACC_MM)
                cb = stp.tile([128, 1], fp32, tag="cb", name="cb")
                nc.scalar.activation(out=cb, in_=acc[:, ao + 88:ao + 89],
                                     func=AF.Identity, bias=b1c)

                nc.vector.tensor_scalar(out=hsb[:, 0:2000],
                                        in0=hsb[:, 0:2000], scalar1=cb,
                                        scalar2=None, op0=OP.add)
                nc.vector.memset(hsb[:, 2000:TPAD], 0.0)

                th = thp.tile([128, TPAD], bf16, tag="th", name="th")
                st["th"] = th
                thT = hp.tile([128, TPAD], bf16, tag="thT", name="thT")
                for g in range(2):   # T-halves of 8 blocks
                    gs = slice(g * 8, (g + 1) * 8)
                    ho = slice(g * 1024, (g + 1) * 1024)
                    tp = pQ.tile([128, 1024], bf16, tag="Q", name="tp")
                    for b in range(g * 8, (g + 1) * 8):
                        bs = slice(b * 128, (b + 1) * 128)
                        ts = slice((b - g * 8) * 128, (b - g * 8 + 1) * 128)
                        nc.tensor.transpose(tp[:, ts], in_=hsb[:, bs],
                                            identity=identb)
                    for b in range(g * 8, (g + 1) * 8):
                        ts = slice((b - g * 8) * 128, (b - g * 8 + 1) * 128)
                        nc.vector.bn_stats(out=st6[:, b, :], in_=tp[:, ts])
                    for b in range(g * 8, (g + 1) * 8):
                        nc.vector.bn_aggr(out=mv[:, b, :], in_=st6[:, b, :])
                    nc.scalar.activation(out=rstd[:, gs], in_=mv[:, gs, 1],
                                         func=AF.Sqrt, bias=eps_c)
                    nc.vector.reciprocal(out=rstd[:, gs], in_=rstd[:, gs])
                    nc.vector.scalar_tensor_tensor(
                        out=nmr[:, gs], in0=mv[:, gs, 0], scalar=-1.0,
                        in1=rstd[:, gs], op0=OP.mult, op1=OP.mult)
                    for b in range(g * 8, (g + 1) * 8):
                        bs = slice(b * 128, (b + 1) * 128)
                        ts = slice((b - g * 8) * 128, (b - g * 8 + 1) * 128)
                        if trivial_ln:
                            nc.scalar.activation(out=thT[:, bs],
                                                 in_=tp[:, ts], func=AF.Tanh,
                                                 scale=rstd[:, b:b + 1],
                                                 bias=nmr[:, b:b + 1])
                        else:
                            zb = scr.tile([128, 128], bf16, tag="zb",
                                          name="zb")
                            nc.scalar.activation(out=zb, in_=tp[:, ts],
                                                 func=AF.Identity,
                                                 scale=rstd[:, b:b + 1],
                                                 bias=nmr[:, b:b + 1])
                            nc.vector.tensor_mul(out=zb, in0=zb, in1=gb)
                            nc.vector.tensor_add(out=zb, in0=zb, in1=bb)
                            nc.scalar.activation(out=thT[:, bs], in_=zb,
                                                 func=AF.Tanh)
                    tpb = pQ.tile([128, 1024], bf16, tag="Q", name="tpb")
                    for b in range(g * 8, (g + 1) * 8):
                        bs = slice(b * 128, (b + 1) * 128)
                        ts = slice((b - g * 8) * 128, (b - g * 8 + 1) * 128)
                        nc.tensor.transpose(tpb[:, ts], in_=thT[:, bs],
                                            identity=identb)
                    # PSUM->SBUF copy doubling as the relu:
                    # th = max(tanh(z), 0) == tanh(relu(z))
                    eng = nc.gpsimd if (POOL_COPY and g == 1) else nc.vector
                    eng.tensor_scalar(out=th[:, ho], in0=tpb,
                                      scalar1=0.0, scalar2=None, op0=OP.max)

            def emit_back(s, feeders=None):
                """Phase 4 per T-block with LAG-2 matvec groups; `feeders`
                maps block index -> emit thunks for the NEXT sample's front
                (interleaved into this sample's engine streams)."""
                st = S[s]
                ao = 96 * s
                xm, th, sv = st["xm"], st["th"], st["sv"]
                Lc = st["sc"][:, 0:1]
                L2c = st["sc"][:, 1:2]
                rden = sv[:, 48:60]
                mu2 = sv[:, 60:72]
                ms2 = sv[:, 72:84]
                sd2 = sv[:, 84:96]
                t0_a = sv[:, 96:108]

                LAG = 2        # den/sex matvec groups (DVE-produced)
                LAG2 = 4       # sx2 groups (p2 may come from slow GPSIMD)

                def emit_matvecs(item):
                    b_, eT_, p1_ = item
                    for i in range(NCH):
                        js = slice(i * 128, (i + 1) * 128)
                        nc.tensor.matmul(acc[:, ao + 32 + i:ao + 33 + i],
                                         lhsT=eT_[:, js], rhs=ones_col,
                                         **ACC_MM)
                        nc.tensor.matmul(acc[:, ao + 44 + i:ao + 45 + i],
                                         lhsT=p1_[:, js], rhs=ones_col,
                                         **ACC_MM)

                def emit_matvecs2(item):
                    b_, p2_ = item
                    for i in range(NCH):
                        js = slice(i * 128, (i + 1) * 128)
                        nc.tensor.matmul(acc[:, ao + 56 + i:ao + 57 + i],
                                         lhsT=p2_[:, js], rhs=ones_col,
                                         **ACC_MM)

                pending = []
                pending2 = []
                for b in range(NTT):
                    bs = slice(b * 128, (b + 1) * 128)
                    aps = pAU.tile([128, 1536], fp32, tag="AU", name="aps")
                    for kb in range(3):
                        ks = slice(kb * 512, (kb + 1) * 512)
                        if trivial_b2:
                            nc.tensor.matmul(aps[:, ks], lhsT=th[:, bs],
                                             rhs=w2T[:, ks],
                                             start=True, stop=True)
                        else:
                            nc.tensor.matmul(aps[:, ks], lhsT=ones1,
                                             rhs=b2r[:, ks],
                                             start=True, stop=False)
                            nc.tensor.matmul(aps[:, ks], lhsT=th[:, bs],
                                             rhs=w2T[:, ks],
                                             start=False, stop=True)
                    eT = scr.tile([128, C], bf16, tag="e", name="eT")
                    nc.scalar.activation(out=eT, in_=aps, func=AF.Exp,
                                         bias=st["mb"][:, b:b + 1])
                    p1 = scr.tile([128, C], bf16, tag="p1", name="p1")
                    nc.vector.tensor_mul(out=p1, in0=eT, in1=xm[b])
                    p2 = scr.tile([128, C], bf16, tag="p2", name="p2", bufs=5)
                    p2eng = (nc.gpsimd if (POOL_P2_EVERY
                                           and b % POOL_P2_EVERY == 0)
                             else nc.vector)
                    p2eng.tensor_mul(out=p2, in0=p1, in1=xm[b])
                    pending.append((b, eT, p1))
                    pending2.append((b, p2))
                    if len(pending) > LAG:
                        emit_matvecs(pending.pop(0))
                    if len(pending2) > LAG2:
                        emit_matvecs2(pending2.pop(0))
                    if feeders and b in feeders:
                        for thunk in feeders[b]:
                            thunk()
                for item in pending:
                    emit_matvecs(item)
                for item in pending2:
                    emit_matvecs2(item)

                # outputs: mu2 = L*sex'/den, ms2 = L^2*sx2'/den - mu2^2
                nc.vector.reciprocal(out=rden, in_=acc[:, ao + 32:ao + 44])
                nc.vector.tensor_mul(out=mu2, in0=acc[:, ao + 44:ao + 56],
                                     in1=rden)
                nc.vector.tensor_scalar(out=mu2, in0=mu2, scalar1=Lc,
                                        scalar2=None, op0=OP.mult)
                nc.vector.tensor_mul(out=ms2, in0=acc[:, ao + 56:ao + 68],
                                     in1=rden)
                nc.vector.tensor_scalar(out=ms2, in0=ms2, scalar1=L2c,
                                        scalar2=None, op0=OP.mult)
                nc.vector.tensor_mul(out=t0_a, in0=mu2, in1=mu2)
                nc.vector.tensor_tensor(out=ms2, in0=ms2, in1=t0_a,
                                        op=OP.subtract)
                nc.vector.tensor_scalar(out=ms2, in0=ms2, scalar1=1e-9,
                                        scalar2=None, op0=OP.max)
                nc.scalar.activation(out=sd2, in_=ms2, func=AF.Sqrt)

                nc.sync.dma_start(
                    out=out_d[s, 0:C].rearrange("(i p) -> p i", p=128),
                    in_=mu2)
                nc.sync.dma_start(
                    out=out_d[s, C:2 * C].rearrange("(i p) -> p i", p=128),
                    in_=sd2)

            # ---------------- emission schedule ----------------
            emit_dmas(0)
            emit_dmas(1)
            emit_fA(0)
            for b in range(NTT):
                emit_fB(0, b)
            emit_fC(0)
            # sample 1's front interleaved into sample 0's phase 4:
            #   slot 6: mm1 (its x chunks land by then)
            #   slots 7..15: mu matvecs per block, arrival-matched
            #   after the loop: Gram q, then the LN chain
            feeders = {8: [lambda: emit_fA(1)]}
            for b2 in range(NTT):
                slot = max(0, (b2 * 7) // 16 - 1)
                feeders.setdefault(slot, []).append(
                    lambda b2=b2: emit_fB(1, b2))
            feeders.setdefault(12, []).append(lambda: emit_fC(1))
            emit_back(0, feeders)
            emit_back(1)

    _split_waits(nc)
    return nc


def _prep_weights(w1, b1, ln_g, ln_b, w2, b2):
    import ml_dtypes
    f = np.float32
    bf = ml_dtypes.bfloat16
    w1T = np.ascontiguousarray(np.asarray(w1, f).T)      # [3C, 128]
    f8 = ml_dtypes.float8_e4m3
    w1aT = np.ascontiguousarray(
        w1T[0:C].reshape(NCH, 128, 128).transpose(1, 0, 2)).astype(f8)
    w1bT = np.ascontiguousarray(
        w1T[C:2 * C].reshape(NCH, 128, 128).transpose(1, 0, 2)).astype(bf)
    w1cT = np.ascontiguousarray(
        w1T[2 * C:3 * C].reshape(NCH, 128, 128).transpose(1, 0, 2)).astype(bf)
    w2T = np.ascontiguousarray(np.asarray(w2, f).T).astype(bf)   # [128, C]
    cst = np.zeros((128, 16), f)
    cst[:, 0] = np.asarray(b1, f)
    cst[:, 13] = 1e-5
    gbv = np.tile(np.asarray(ln_g, f)[None, :], (128, 1))
    bbv = np.tile(np.asarray(ln_b, f)[None, :], (128, 1))
    return dict(w1aT=w1aT, w1bT=w1bT, w1cT=w1cT, w2T=w2T, cst=cst,
                ones_col=np.ones((128, 1), bf),
                identb=np.eye(128, dtype=f).astype(bf),
                gb=np.ascontiguousarray(gbv), bb=np.ascontiguousarray(bbv),
                ones1=np.ones((1, 128), bf),
                b2r=np.asarray(b2, f)[None, :].astype(bf))


def kernel(x, mask, w1, b1, ln_g, ln_b, w2, b2, _profile=None):
    from concourse.bass_utils import run_bass_kernel_spmd
    import ml_dtypes
    bf = ml_dtypes.bfloat16

    trivial_ln = bool(np.all(np.asarray(ln_g) == 1.0)
                      and np.all(np.asarray(ln_b) == 0.0))
    trivial_b2 = bool(np.all(np.asarray(b2) == 0.0))
    key = ("nc", trivial_ln, trivial_b2)
    if key not in _CACHE:
        _CACHE[key] = _build_nc(trivial_ln, trivial_b2)
    nc = _CACHE[key]

    wts = _prep_weights(w1, b1, ln_g, ln_b, w2, b2)

    xf = np.asarray(x, np.float32)                       # [B, C, T]
    x_nat = np.ascontiguousarray(xf.astype(ml_dtypes.float8_e4m3))

    mf = np.asarray(mask, np.float32).reshape(B, T)
    L = mf.sum(axis=1)                                   # valid frames
    nm = np.zeros((B, TPAD), np.float32)
    nm[:, :T] = mf / L[:, None]
    xmv = np.zeros((B, TPAD, C), np.float32)
    xmv[:, :T, :] = xf.transpose(0, 2, 1)
    xmv *= nm[:, :, None]
    xm = np.ascontiguousarray(xmv.reshape(B, NTT, 128, C).astype(bf))

    mbT = np.full((B, TPAD), MASK_NEG, np.float32)
    mbT[:, :T] = (mf - 1.0) * -MASK_NEG
    mbT = np.ascontiguousarray(mbT.reshape(B, NTT, 128).transpose(0, 2, 1))
    scal = np.zeros((B, 128, 2), np.float32)
    scal[:, :, 0] = L[:, None]
    scal[:, :, 1] = (L * L)[:, None]

    in_maps = []
    for c in range(NCORES):
        sl = slice(c * BPC, (c + 1) * BPC)
        m = {"x": x_nat[sl], "xm": xm[sl], "mbT": mbT[sl], "scal": scal[sl]}
        m.update(wts)
        in_maps.append(m)

    kw = dict(_profile) if _profile else {}
    res = run_bass_kernel_spmd(nc, in_maps, list(range(NCORES)), **kw)
    out = np.concatenate([res.results[c]["out"] for c in range(NCORES)], axis=0)
    if _profile:
        _CACHE["last_result"] = res
    return out.reshape(B, 2 * C)
